# revision 1
# baseline (speedup 1.0000x reference)
"""Causal multi-head attention (B=2, T=2048, C=1024, H=16, D=64) on 8 TRN2 cores.

Sharding: 2 heads per core (head-parallel). Each core:
  qkvT = W_slice.T @ xT            [384, 4096]   (Q/K/V x 2 heads, dims on partitions)
  scoresT[k, q] = K @ Q.T / 8      per 128-key block, per 512-query tile (causal-skipped)
  attT = exp(scoresT) * tri-mask   (softmax numerator, keys on partitions)
  accT = [V | 1].T @ attT          -> [65, 512]: rows 0-63 numerator.T, row 64 denominator
  yT = accT[0:64] * recip(accT[64]) (gpsimd partition-broadcast of the reciprocal row)
  partial = yT.T @ W_proj_rows     [4096, 1024]
Host sums the 8 partials. All matmuls run in float32r (full-rate fp32, ~1e-3 ulp).
"""
import sys

sys.path.insert(0, "/opt/trn_rl_repo")

import numpy as np

import concourse.bass as bass
import concourse.mybir as mybir
from concourse import bacc
from concourse.tile import TileContext
from concourse.bass_utils import run_bass_kernel_spmd

N_CORES = 8
B, T, C = 2, 2048, 1024
D = 64          # head dim
NH = 2          # heads per core
HC = NH * D     # 128: head-channels per core
BT = B * T      # 4096
TQ = 512        # query tile
NM = T // TQ    # 4 query tiles per batch
NKB = T // 128  # 16 key blocks per batch
F32 = mybir.dt.float32
F32R = mybir.dt.float32r
SCALE = 1.0 / np.sqrt(D)  # 0.125


def build_program(trace_scopes: bool = False):
    nc = bacc.Bacc("TRN2", target_bir_lowering=False, debug=False)

    xT = nc.dram_tensor("xT", [C, BT], F32R, kind="ExternalInput")
    wk = nc.dram_tensor("wk", [C, 3 * HC], F32R, kind="ExternalInput")
    wp = nc.dram_tensor("wp", [HC, C], F32R, kind="ExternalInput")
    zt = nc.dram_tensor("zt", [128, 4, 128], F32R, kind="ExternalInput")
    ident = nc.dram_tensor("ident", [128, 128], F32R, kind="ExternalInput")
    po = nc.dram_tensor("po", [BT, C], F32, kind="ExternalOutput")

    xT3 = xT.ap().rearrange("(o p) t -> p o t", p=128)   # [128, 8, 4096]
    wk3 = wk.ap().rearrange("(o p) m -> p o m", p=128)   # [128, 8, 384]

    with TileContext(nc) as tc:
        with (
            tc.tile_pool(name="consts", bufs=1) as consts,
            tc.tile_pool(name="xin", bufs=3) as xin,
            tc.tile_pool(name="qkv", bufs=1) as qkvp,
            tc.tile_pool(name="vext", bufs=1) as vextp,
            tc.tile_pool(name="att", bufs=4) as attp,
            tc.tile_pool(name="yt", bufs=1) as ytp,
            tc.tile_pool(name="oout", bufs=3) as outp,
            tc.tile_pool(name="nrm", bufs=2) as nrmp,
        ):
            wq_sb = consts.tile([128, 8, 3 * HC], F32R)
            nc.sync.dma_start(wq_sb[:], wk3)
            wp_sb = consts.tile([HC, C], F32R)
            zt_sb = consts.tile([128, 4, 128], F32R)
            id_sb = consts.tile([128, 128], F32R)
            ones_sb = consts.tile([128, 1], F32)
            nc.gpsimd.memset(ones_sb[:], 1.0)

            def load_consts():
                # issued after the first x block so the critical-path DMAs
                # (wq + xblk0) reach the SP queue first
                nc.sync.dma_start(wp_sb[:], wp.ap())
                nc.sync.dma_start(zt_sb[:], zt.ap())
                nc.sync.dma_start(id_sb[:], ident.ap())

            for b in range(B):
                # ---------------- QKV projection for batch b ----------------
                QT = qkvp.tile([128, T], F32R, tag="QT")
                KT = qkvp.tile([128, T], F32R, tag="KT")
                VT = qkvp.tile([128, T], F32R, tag="VT")
                dsts = [QT, KT, VT]
                V_ext = vextp.tile([128, NH, NKB, D + 1], F32R, tag="vext")
                nc.vector.tensor_copy(
                    V_ext[:, :, :, D : D + 1],
                    ones_sb[:].to_broadcast((128, NH, NKB, 1)),
                )
                with (
                    tc.tile_pool(name="qkvps", bufs=3, space="PSUM") as qkvps,
                    tc.tile_pool(name="vtps", bufs=2, space="PSUM") as vtps,
                ):
                    for tb in range(4):
                        xblk = xin.tile([128, 8, TQ], F32R, tag="xblk")
                        c0 = b * T + tb * TQ
                        nc.sync.dma_start(xblk[:], xT3[:, :, c0 : c0 + TQ])
                        if b == 0 and tb == 0:
                            load_consts()
                        for mt in range(3):
                            ps = qkvps.tile([128, TQ], F32, tag="qk")
                            for kt in range(8):
                                nc.tensor.matmul(
                                    ps[:],
                                    wq_sb[:, kt, mt * 128 : (mt + 1) * 128],
                                    xblk[:, kt, :],
                                    start=(kt == 0),
                                    stop=(kt == 7),
                                )
                            nc.any.tensor_copy(
                                dsts[mt][:, tb * TQ : (tb + 1) * TQ], ps[:]
                            )
                        # V transpose (+ ones col done above) for this block
                        for k4 in range(4):
                            kb = tb * 4 + k4
                            pt = vtps.tile([128, 128], F32R, tag="vt")
                            nc.tensor.transpose(
                                pt[:], VT[:, kb * 128 : (kb + 1) * 128], id_sb[:]
                            )
                            nc.vector.tensor_copy(
                                V_ext[:, :, kb, 0:D],
                                pt.rearrange("p (h d) -> p h d", h=NH),
                            )

                # ---------------- attention + projection ----------------
                yT = ytp.tile([HC, T], F32R, tag="yT")
                with (
                    tc.tile_pool(name="pw", bufs=2, space="PSUM") as pwps,
                    tc.tile_pool(name="acc", bufs=2, space="PSUM") as accps,
                    tc.tile_pool(name="pj", bufs=1, space="PSUM") as pjps,
                ):
                    for m in range(NM):
                        q0 = m * TQ
                        for h in range(NH):
                            hs = slice(h * D, (h + 1) * D)
                            acc = accps.tile([D + 1, TQ], F32, tag="acc")
                            ng = 2 * (m + 1)  # kb pairs
                            for g in range(ng):
                                pw = pwps.tile([128, 2, TQ], F32, tag="pw")
                                for j2 in range(2):
                                    kb = 2 * g + j2
                                    nc.tensor.matmul(
                                        pw[:, j2],
                                        KT[hs, kb * 128 : (kb + 1) * 128],
                                        QT[hs, q0 : q0 + TQ],
                                        start=True,
                                        stop=True,
                                    )
                                attT = attp.tile([128, 2, TQ], F32R, tag="attT")
                                nc.scalar.activation(
                                    attT[:],
                                    pw[:],
                                    mybir.ActivationFunctionType.Exp,
                                    scale=float(SCALE),
                                )
                                for j2 in range(2):
                                    kb = 2 * g + j2
                                    jj = kb - 4 * m
                                    if jj >= 0:
                                        # triangular mask on the diagonal block
                                        ds = slice(jj * 128, (jj + 1) * 128)
                                        nc.vector.tensor_tensor(
                                            attT[:, j2, ds],
                                            attT[:, j2, ds],
                                            zt_sb[:, 3, :],
                                            mybir.AluOpType.mult,
                                        )
                                for j2 in range(2):
                                    kb = 2 * g + j2
                                    jj = kb - 4 * m
                                    q_lo = max(jj, 0) * 128  # cols < q_lo are masked
                                    nc.tensor.matmul(
                                        acc[:, q_lo:TQ],
                                        V_ext[:, h, kb, :],
                                        attT[:, j2, q_lo:TQ],
                                        start=(g == 0 and j2 == 0),
                                        stop=(g == ng - 1 and j2 == 1),
                                    )
                            # normalize: yT[hs, q-slice] = num / den
                            d_sb = nrmp.tile([1, TQ], F32, tag="d_sb")
                            nc.vector.tensor_copy(d_sb[:], acc[D : D + 1, :])
                            r = nrmp.tile([1, TQ], F32, tag="r")
                            nc.vector.reciprocal_approx_fast(r[:], d_sb[:])
                            rb = nrmp.tile([D, TQ], F32, tag="rb")
                            nc.gpsimd.partition_broadcast(rb[:], r[:])
                            nc.vector.tensor_tensor(
                                yT[hs, q0 : q0 + TQ],
                                acc[0:D, :],
                                rb[:],
                                mybir.AluOpType.mult,
                            )
                        # out-projection for this query tile
                        for t4 in range(4):
                            t0 = q0 + t4 * 128
                            pj = pjps.tile([128, C], F32, tag="pj")
                            for nn in range(2):
                                nc.tensor.matmul(
                                    pj[:, nn * 512 : (nn + 1) * 512],
                                    yT[:, t0 : t0 + 128],
                                    wp_sb[:, nn * 512 : (nn + 1) * 512],
                                    start=True,
                                    stop=True,
                                )
                            ot = outp.tile([128, C], F32, tag="ot")
                            nc.any.tensor_copy(ot[:], pj[:])
                            nc.sync.dma_start(
                                po.ap()[b * T + t0 : b * T + t0 + 128, :], ot[:]
                            )

    nc.compile()
    return nc


_CACHED_NC = None


def kernel(x: np.ndarray, w_qkv: np.ndarray, w_proj: np.ndarray) -> np.ndarray:
    global _CACHED_NC
    if _CACHED_NC is None:
        _CACHED_NC = build_program()
    nc = _CACHED_NC

    x = np.ascontiguousarray(x, dtype=np.float32)
    w_qkv = np.ascontiguousarray(w_qkv, dtype=np.float32)
    w_proj = np.ascontiguousarray(w_proj, dtype=np.float32)

    xT = np.ascontiguousarray(x.reshape(BT, C).T)  # [C, BT]
    zt = np.zeros((128, 4, 128), dtype=np.float32)
    zt[:, 3] = np.triu(np.ones((128, 128), dtype=np.float32))
    ident = np.eye(128, dtype=np.float32)

    in_maps = []
    for i in range(N_CORES):
        cs = slice(HC * i, HC * (i + 1))
        wk_i = np.ascontiguousarray(
            np.concatenate(
                [w_qkv[:, cs], w_qkv[:, C:][:, cs], w_qkv[:, 2 * C :][:, cs]], axis=1
            )
        )
        wp_i = np.ascontiguousarray(w_proj[cs, :])
        in_maps.append(
            {"xT": xT, "wk": wk_i, "wp": wp_i, "zt": zt, "ident": ident}
        )

    res = run_bass_kernel_spmd(nc, in_maps, core_ids=list(range(N_CORES)))
    total = np.zeros((BT, C), dtype=np.float64)
    for i in range(N_CORES):
        total += res.results[i]["po"]
    return total.astype(np.float32).reshape(B, T, C)


if __name__ == "__main__":
    rng = np.random.default_rng(0)
    x = rng.standard_normal((B, T, C), dtype=np.float32)
    w_qkv = rng.standard_normal((C, 3 * C), dtype=np.float32) / np.sqrt(C)
    w_proj = rng.standard_normal((C, C), dtype=np.float32) / np.sqrt(C)
    out = kernel(x=x, w_qkv=w_qkv, w_proj=w_proj)
    print(out.shape, out.dtype, np.abs(out).mean())

